# revision 30
# baseline (speedup 1.0000x reference)
"""Per-pixel adaptive (kernel-prediction) 5x5 conv on 8 trn2 cores.

out[b,c,y,x] = sum_{i,j} x_pad[b,c,y+i,x+j] * kernel[b,(c*5+i)*5+j,y,x]
with edge (replication) padding p=2.

Sharding: 8 cores = B(4) x C-halves(2).  The op is depthwise (output
channel c reads only input channel c), so slicing C needs no halo.

Device layout: 128 SBUF partitions = 16 channels x 8 row-groups of 32
rows.  Rows are processed in 4 quarter-passes of 8 rows (2048 output
elems per partition per quarter = 4 PSUM banks, so two quarters
ping-pong in PSUM).

The kernel tensor dominates HBM traffic (52.4 MB f16 per core).  It is
pre-swizzled ON THE HOST into [20 chunks][128 partitions][5 taps x 2048]
so each chunk DMA reads one 20 KB contiguous block per partition
(descriptor size 20 KB instead of 8 KB -> much better per-packet DMA
efficiency).  Chunks alternate between the two HWDGE queues (sync /
scalar).

Per tap: DVE computes the f16 product into a tmp tile; the otherwise
idle TensorE accumulates tmp into PSUM via identity matmuls (PSUM
accumulate-on-write does the adds).  ScalarE drains PSUM -> SBUF;
gpsimd SWDGE stores to DRAM.
"""

import numpy as np

B, C, H, W, K = 4, 32, 256, 256, 5
P = (K - 1) // 2   # 2
CP = 16            # channels per core
YG = 8             # row groups per partition-set
RG = H // YG       # 32 rows per group
WP = W + 2 * P     # 260
SROWS = RG + 2 * P  # 36 rows per x stripe
SLEN = SROWS * WP   # 9360 elems per partition x-stripe
NQ = 4             # quarter-passes per row group
QR = RG // NQ      # 8 rows per quarter
QFREE = QR * W     # 2048 free elems per quarter
NTG = 5            # tap groups (of 5 taps each)
CHUNK = NTG * QFREE  # 10240 elems per partition per chunk DMA
NCHUNK = NQ * NTG    # 20 chunk DMAs
XSPLIT = 14 * WP     # x rows 0..13 cover all q=0 taps; rest loads in parallel

# offload the last tap of each chunk to GpSimd's vector path (relieves DVE);
# it is issued first so its ~4.9us hides under the 4 DVE taps (~4.9us)
GP_OFFLOAD = False

_cache = {}


def _build_nc():
    import concourse.bass as bass
    import concourse.tile as tile
    from concourse import bacc, mybir

    f32 = mybir.dt.float32
    f16 = mybir.dt.float16
    nc = bacc.Bacc("TRN2", target_bir_lowering=False, debug=False, num_devices=8)

    xsw_t = nc.dram_tensor("xsw", [128, SLEN], f16, kind="ExternalInput")
    ksw_t = nc.dram_tensor("ksw", [NCHUNK, 128, CHUNK], f16, kind="ExternalInput")
    ident_t = nc.dram_tensor("ident", [128, 128], f16, kind="ExternalInput")
    out_t = nc.dram_tensor("out", [NQ, 128, QFREE], f16, kind="ExternalOutput")

    with tile.TileContext(nc) as tc:
        with (
            tc.tile_pool(name="xp", bufs=1) as xpool,
            tc.tile_pool(name="idp", bufs=1) as ipool,
            tc.tile_pool(name="k0p", bufs=NTG) as k0pool,
            tc.tile_pool(name="kp", bufs=6) as kpool,
            tc.tile_pool(name="tp", bufs=6) as tpool,
            tc.tile_pool(name="op", bufs=8) as opool,
            tc.tile_pool(name="pp", bufs=2, space="PSUM") as ppool,
        ):
            xtile = xpool.tile([128, SLEN], f16)
            ident = ipool.tile([128, 128], f16)

            # startup: the first-needed data (x head + chunk0) is split
            # across BOTH rings so per-packet round-robin can't starve it
            # behind later chunks; the x tail (not needed until chunk 5)
            # rides the idle SWDGE queue.  Chunks 1+ use whole-ring 20KB
            # descriptors (best per-packet efficiency), alternating rings
            # in consumption order.
            XH = XSPLIT // 2
            def emit_x_head():
                # emitted AFTER chunk0's halves so chunk0 heads both rings
                nc.sync.dma_start(
                    out=xtile[:, :XH],
                    in_=bass.AP(xsw_t, 0, [[SLEN, 128], [1, XH]]),
                    single_packet=True,
                )
                nc.scalar.dma_start(out=ident[:], in_=ident_t[:, :])
                nc.scalar.dma_start(
                    out=xtile[:, XH:XSPLIT],
                    in_=bass.AP(xsw_t, XH, [[SLEN, 128], [1, XSPLIT - XH]]),
                    single_packet=True,
                )
                nc.gpsimd.dma_start(
                    out=xtile[:, XSPLIT:],
                    in_=bass.AP(xsw_t, XSPLIT, [[SLEN, 128], [1, SLEN - XSPLIT]]),
                )

            x3 = xtile[:].rearrange("p (r w) -> p r w", w=WP)

            def emit_drain(q, ptile):
                # drain PSUM in 4 pipelined pieces: copy_s -> store_s
                # overlap, shrinking the end-of-kernel tail.  The last
                # quarter's stores use the HWDGE rings (empty by then).
                NS = 4
                SP = QFREE // NS
                for si in range(NS):
                    obuf = opool.tile([128, SP], f16, tag="ob")
                    nc.scalar.copy(obuf[:], ptile[:, si * SP : (si + 1) * SP])
                    dst = bass.AP(
                        out_t,
                        q * 128 * QFREE + si * SP,
                        [[QFREE, 128], [1, SP]],
                    )
                    if q == NQ - 1:
                        oeng = nc.sync if si % 2 == 0 else nc.scalar
                    else:
                        oeng = nc.gpsimd
                    oeng.dma_start(out=dst, in_=obuf[:])

            ASPLIT = 2 * QFREE  # taps 0-1 -> sync ring, taps 2-4 -> scalar
            pending_drains = []
            for chunk in range(NCHUNK):
                q, tg = divmod(chunk, NTG)
                ktile = kpool.tile([128, CHUNK], f16, tag="kt")
                base = chunk * 128 * CHUNK
                if chunk < 2:
                    # first two chunks split across both rings: they lead
                    # both ring FIFOs, so the pipeline primes in ~2 chunk
                    # times with no round-robin starvation.
                    nc.sync.dma_start(
                        out=ktile[:, :ASPLIT],
                        in_=bass.AP(ksw_t, base, [[CHUNK, 128], [1, ASPLIT]]),
                        single_packet=True,
                    )
                    nc.scalar.dma_start(
                        out=ktile[:, ASPLIT:],
                        in_=bass.AP(
                            ksw_t,
                            base + ASPLIT,
                            [[CHUNK, 128], [1, CHUNK - ASPLIT]],
                        ),
                        single_packet=True,
                    )
                    if chunk == 0:
                        emit_x_head()
                else:
                    keng = nc.sync if chunk % 2 == 0 else nc.scalar
                    keng.dma_start(
                        out=ktile[:],
                        in_=bass.AP(ksw_t, base, [[CHUNK, 128], [1, CHUNK]]),
                        single_packet=True,
                    )

                # deferred drains: emitted AFTER this chunk's DMA issue so a
                # drain's sem wait can never block a chunk-DMA issue on the
                # scalar queue.
                while pending_drains and pending_drains[0][0] + 3 <= chunk:
                    emit_drain(*pending_drains.pop(0)[1])

                if tg == 0:
                    ptile = ppool.tile([128, QFREE], f32, tag="ps")

                for t in range(NTG):
                    ij = tg * NTG + t
                    i, j = divmod(ij, K)
                    xv = x3[:, q * QR + i : q * QR + i + QR, j : j + W]
                    k3 = ktile[:, t * QFREE : (t + 1) * QFREE].rearrange(
                        "p (r w) -> p r w", w=W
                    )
                    tmp = tpool.tile([128, QFREE], f16, tag="tmp")
                    t3 = tmp[:].rearrange("p (r w) -> p r w", w=W)
                    nc.vector.tensor_mul(t3, xv, k3)
                    for bk in range(QFREE // 512):
                        nc.tensor.matmul(
                            out=ptile[:, bk * 512 : (bk + 1) * 512],
                            lhsT=ident[:],
                            rhs=tmp[:, bk * 512 : (bk + 1) * 512],
                            start=(ij == 0),
                            stop=(ij == K * K - 1),
                        )

                if tg == NTG - 1:
                    pending_drains.append((chunk, (q, ptile)))

            for _, args in pending_drains:
                emit_drain(*args)

    nc.compile()
    return nc


def _get_nc():
    if "nc" not in _cache:
        _cache["nc"] = _build_nc()
    return _cache["nc"]


_IDENT = np.eye(128, dtype=np.float16)

# row index grid for the overlapping padded x stripes: (YG, SROWS)
_ROWS = (np.arange(YG)[:, None] * RG + np.arange(SROWS)[None, :])


def prepare_in_maps(x, kern):
    """Host-side shard + swizzle.  x, kern: full f32 arrays."""
    x = np.asarray(x, dtype=np.float32).astype(np.float16)
    kern = np.asarray(kern, dtype=np.float32).astype(np.float16)
    xpad = np.pad(x, ((0, 0), (0, 0), (P, P), (P, P)), mode="edge")

    in_maps = []
    for core in range(8):
        b, half = divmod(core, 2)
        c0 = half * CP
        # x stripes: (CP, YG, SROWS, WP) -> (128, SLEN)
        xs = xpad[b, c0 : c0 + CP][:, _ROWS, :].reshape(128, SLEN)
        # kernel swizzle: (CP*K*K, H, W) ->
        # (c, tg, t, g, q, r, w) -> (q, tg, c, g, t, r, w) -> (20, 128, CHUNK)
        kc = kern[b, c0 * K * K : (c0 + CP) * K * K].reshape(
            CP, NTG, K, YG, NQ, QR, W
        )
        ks = np.ascontiguousarray(kc.transpose(4, 1, 0, 3, 2, 5, 6)).reshape(
            NCHUNK, 128, CHUNK
        )
        in_maps.append({"xsw": np.ascontiguousarray(xs), "ksw": ks, "ident": _IDENT})
    return in_maps


def kernel(x, kernel, kernel_size):
    from concourse.bass_utils import run_bass_kernel_spmd

    in_maps = prepare_in_maps(x, kernel)
    nc = _get_nc()
    res = run_bass_kernel_spmd(nc, in_maps, list(range(8)))

    out = np.empty((B, C, H, W), dtype=np.float32)
    for core in range(8):
        b, half = divmod(core, 2)
        c0 = half * CP
        # out_sw: (NQ, 128, QFREE) -> (q, c, g, r, w) -> (c, g, q, r, w)
        osw = res.results[core]["out"].reshape(NQ, CP, YG, QR, W)
        out[b, c0 : c0 + CP] = (
            osw.transpose(1, 2, 0, 3, 4).reshape(CP, H, W).astype(np.float32)
        )
    return out


# revision 31
# speedup vs baseline: 1.1632x; 1.1632x over previous
"""Per-pixel adaptive (kernel-prediction) 5x5 conv on 8 trn2 cores.

out[b,c,y,x] = sum_{i,j} x_pad[b,c,y+i,x+j] * kernel[b,(c*5+i)*5+j,y,x]
with edge (replication) padding p=2.

Sharding: 8 cores = B(4) x C-halves(2).  The op is depthwise (output
channel c reads only input channel c), so slicing C needs no halo.

Device layout: 128 SBUF partitions = 16 channels x 8 row-groups of 32
rows.  Rows are processed in 4 quarter-passes of 8 rows (2048 output
elems per partition per quarter = 4 PSUM banks, so two quarters
ping-pong in PSUM).

The kernel tensor dominates HBM traffic (52.4 MB f16 per core).  It is
pre-swizzled ON THE HOST into [20 chunks][128 partitions][5 taps x 2048]
so each chunk DMA reads one 20 KB contiguous block per partition
(descriptor size 20 KB instead of 8 KB -> much better per-packet DMA
efficiency).  Chunks alternate between the two HWDGE queues (sync /
scalar).

Per tap: DVE computes the f16 product into a tmp tile; the otherwise
idle TensorE accumulates tmp into PSUM via identity matmuls (PSUM
accumulate-on-write does the adds).  ScalarE drains PSUM -> SBUF;
gpsimd SWDGE stores to DRAM.
"""

import numpy as np

B, C, H, W, K = 4, 32, 256, 256, 5
P = (K - 1) // 2   # 2
CP = 16            # channels per core
YG = 8             # row groups per partition-set
RG = H // YG       # 32 rows per group
WP = W + 2 * P     # 260
SROWS = RG + 2 * P  # 36 rows per x stripe
SLEN = SROWS * WP   # 9360 elems per partition x-stripe
NQ = 4             # quarter-passes per row group
QR = RG // NQ      # 8 rows per quarter
QFREE = QR * W     # 2048 free elems per quarter
NTG = 5            # tap groups (of 5 taps each)
CHUNK = NTG * QFREE  # 10240 elems per partition per chunk DMA
NCHUNK = NQ * NTG    # 20 chunk DMAs

_cache = {}


def _build_nc():
    import concourse.bass as bass
    import concourse.tile as tile
    from concourse import bacc, mybir

    f32 = mybir.dt.float32
    f16 = mybir.dt.float16
    nc = bacc.Bacc("TRN2", target_bir_lowering=False, debug=False, num_devices=8)

    xsw_t = nc.dram_tensor("xsw", [128, SLEN], f16, kind="ExternalInput")
    ksw_t = nc.dram_tensor("ksw", [NCHUNK, 128, CHUNK], f16, kind="ExternalInput")
    ident_t = nc.dram_tensor("ident", [128, 128], f16, kind="ExternalInput")
    out_t = nc.dram_tensor("out", [NQ, 128, QFREE], f16, kind="ExternalOutput")

    with tile.TileContext(nc) as tc:
        with (
            tc.tile_pool(name="xp", bufs=1) as xpool,
            tc.tile_pool(name="idp", bufs=1) as ipool,
            tc.tile_pool(name="kp", bufs=6) as kpool,
            tc.tile_pool(name="tp", bufs=6) as tpool,
            tc.tile_pool(name="op", bufs=2) as opool,
            tc.tile_pool(name="pp", bufs=2, space="PSUM") as ppool,
        ):
            xtile = xpool.tile([128, SLEN], f16)
            nc.sync.dma_start(
                out=xtile[:],
                in_=bass.AP(xsw_t, 0, [[SLEN, 128], [1, SLEN]]),
                single_packet=True,
            )
            ident = ipool.tile([128, 128], f16)
            nc.scalar.dma_start(out=ident[:], in_=ident_t[:, :])

            x3 = xtile[:].rearrange("p (r w) -> p r w", w=WP)

            for chunk in range(NCHUNK):
                q, tg = divmod(chunk, NTG)
                ktile = kpool.tile([128, CHUNK], f16, tag="kt")
                ksrc = bass.AP(
                    ksw_t, chunk * 128 * CHUNK, [[CHUNK, 128], [1, CHUNK]]
                )
                keng = nc.sync if chunk % 2 == 0 else nc.scalar
                keng.dma_start(out=ktile[:], in_=ksrc, single_packet=True)

                if tg == 0:
                    ptile = ppool.tile([128, QFREE], f32, tag="ps")

                for t in range(NTG):
                    ij = tg * NTG + t
                    i, j = divmod(ij, K)
                    xv = x3[:, q * QR + i : q * QR + i + QR, j : j + W]
                    k3 = ktile[:, t * QFREE : (t + 1) * QFREE].rearrange(
                        "p (r w) -> p r w", w=W
                    )
                    tmp = tpool.tile([128, QFREE], f16, tag="tmp")
                    t3 = tmp[:].rearrange("p (r w) -> p r w", w=W)
                    nc.vector.tensor_mul(t3, xv, k3)
                    for bk in range(QFREE // 512):
                        nc.tensor.matmul(
                            out=ptile[:, bk * 512 : (bk + 1) * 512],
                            lhsT=ident[:],
                            rhs=tmp[:, bk * 512 : (bk + 1) * 512],
                            start=(ij == 0),
                            stop=(ij == K * K - 1),
                        )

                if tg == NTG - 1:
                    obuf = opool.tile([128, QFREE], f16, tag="ob")
                    nc.scalar.copy(obuf[:], ptile[:])
                    dst = bass.AP(
                        out_t, q * 128 * QFREE, [[QFREE, 128], [1, QFREE]]
                    )
                    nc.gpsimd.dma_start(out=dst, in_=obuf[:])

    nc.compile()
    return nc


def _get_nc():
    if "nc" not in _cache:
        _cache["nc"] = _build_nc()
    return _cache["nc"]


_IDENT = np.eye(128, dtype=np.float16)

# row index grid for the overlapping padded x stripes: (YG, SROWS)
_ROWS = (np.arange(YG)[:, None] * RG + np.arange(SROWS)[None, :])


def prepare_in_maps(x, kern):
    """Host-side shard + swizzle.  x, kern: full f32 arrays."""
    x = np.asarray(x, dtype=np.float32).astype(np.float16)
    kern = np.asarray(kern, dtype=np.float32).astype(np.float16)
    xpad = np.pad(x, ((0, 0), (0, 0), (P, P), (P, P)), mode="edge")

    in_maps = []
    for core in range(8):
        b, half = divmod(core, 2)
        c0 = half * CP
        # x stripes: (CP, YG, SROWS, WP) -> (128, SLEN)
        xs = xpad[b, c0 : c0 + CP][:, _ROWS, :].reshape(128, SLEN)
        # kernel swizzle: (CP*K*K, H, W) ->
        # (c, tg, t, g, q, r, w) -> (q, tg, c, g, t, r, w) -> (20, 128, CHUNK)
        kc = kern[b, c0 * K * K : (c0 + CP) * K * K].reshape(
            CP, NTG, K, YG, NQ, QR, W
        )
        ks = np.ascontiguousarray(kc.transpose(4, 1, 0, 3, 2, 5, 6)).reshape(
            NCHUNK, 128, CHUNK
        )
        in_maps.append({"xsw": np.ascontiguousarray(xs), "ksw": ks, "ident": _IDENT})
    return in_maps


def kernel(x, kernel, kernel_size):
    from concourse.bass_utils import run_bass_kernel_spmd

    in_maps = prepare_in_maps(x, kernel)
    nc = _get_nc()
    res = run_bass_kernel_spmd(nc, in_maps, list(range(8)))

    out = np.empty((B, C, H, W), dtype=np.float32)
    for core in range(8):
        b, half = divmod(core, 2)
        c0 = half * CP
        # out_sw: (NQ, 128, QFREE) -> (q, c, g, r, w) -> (c, g, q, r, w)
        osw = res.results[core]["out"].reshape(NQ, CP, YG, QR, W)
        out[b, c0 : c0 + CP] = (
            osw.transpose(1, 2, 0, 3, 4).reshape(CP, H, W).astype(np.float32)
        )
    return out
